# revision 1
# baseline (speedup 1.0000x reference)
"""Angles2BasisDihedral Trainium2 kernel (8 NeuronCores, data-parallel).

Math: per sample b with angles alpha/beta (L=512), per-position rotation
  A_j = Rz(alpha_j) @ Rx(beta_j)  (3x3), cumulative M_p = A_1 @ ... @ A_p,
  output[b, r, 3p+c] = M_p[r][c] for p=0..L (M_0 = I), zeroed for p > len_b.

Device strategy (per core, 2048 samples = 16 blocks of 128 partitions):
  - host pre-wraps angles into [-pi, pi] (ACT Sin table is only accurate there)
    and pre-sorts samples by length, striped across the 8 cores; per-block
    chunk counts are baked into the instruction stream at build time.
  - masking is folded into the A entries: A_j = 0 for j > len_b, which makes
    every masked prefix product exactly zero.
  - chunked scan: C<=16 chunks of S=32 along the chain; phase 1 computes local
    prefixes for all chunks in parallel (batch in partitions, chunks in the
    free dim), phase 2 ripples the 3x3 carries across chunks, phase 3 applies
    carries to all local prefixes and writes rows in the final output layout.
  - engine-linear dataflow DMA -> ACT -> DVE -> DMA keeps every instruction at
    <=1 cross-engine semaphore wait (TRN2 ISA limit).
"""
import math
import os
import numpy as np

ABLATE = os.environ.get("KERNEL_ABLATE", "")

import concourse.bacc as bacc
import concourse.mybir as mybir
from concourse.bass_utils import run_bass_kernel_spmd
from concourse.tile import TileContext

B, L = 16384, 512
NCORES = 8
BPC = B // NCORES            # samples per core (2048)
NBLK = BPC // 128            # 16 blocks of 128 partitions
OUTW = 3 * (L + 1)           # 1539 columns per row
INW = 4 * L + 8              # wa | wac | wb | wbc | len+0.5 | pad


def _plan_block(maxlen):
    """(S, C) for this block's max length. Narrow chained DVE ops are far more
    expensive on silicon than the naive 58+FD model, so keep S=32 fixed."""
    if maxlen <= 0:
        return (32, 0)
    return (32, -(-maxlen // 32))
F32 = mybir.dt.float32
ADD = mybir.AluOpType.add
MULT = mybir.AluOpType.mult

LAST_EXEC_NS = None
_CACHE = {}


def _build(plans):
    """Build the Bass program. `plans` is a list of (S, C) per block."""
    nc = bacc.Bacc("TRN2", target_bir_lowering=False)
    # const needed for activation scale=-1.0
    t = nc.alloc_sbuf_tensor("const-f32-neg1", [128, 1], F32)
    nc.gpsimd.memset(t.ap(), -1.0)
    nc.const_aps.aps[(F32, -1.0)] = t.ap()
    nc.all_engine_barrier()

    inp = nc.declare_dram_parameter("inp", [BPC, INW], F32, isOutput=False)
    cst = nc.declare_dram_parameter("cst", [128, L + 9], F32, isOutput=False)
    out = nc.declare_dram_parameter("out", [BPC, 3 * OUTW], F32, isOutput=True)

    SIN = mybir.ActivationFunctionType.Sin
    SIGN = mybir.ActivationFunctionType.Sign
    RELU = mybir.ActivationFunctionType.Relu
    IDT = mybir.ActivationFunctionType.Identity

    with TileContext(nc) as tc:
        with (
            tc.tile_pool(name="pcst", bufs=1) as pcst,
            tc.tile_pool(name="pin", bufs=2) as pin,
            tc.tile_pool(name="ptrig", bufs=2) as ptrig,
            tc.tile_pool(name="pwork", bufs=1) as pwork,
            tc.tile_pool(name="pout", bufs=2) as pout,
        ):
            iota = pcst.tile([128, L + 9], F32)
            nc.gpsimd.dma_start(out=iota[:, :], in_=cst[:, :])
            # ACT warmup: absorb the const-DMA semaphore into ACT's clock
            warm = pcst.tile([128, 1], F32)
            nc.scalar.activation(warm[:, :], iota[:, 0:1], IDT)
            # zero tile: tails/identity are DMA'd to DRAM, not memset on DVE
            zt = pcst.tile([128, 1536], F32)
            nc.vector.memset(zt[:, :], 0.0)

            for b in range(NBLK):
                S, cb = plans[b]
                NP = cb * S
                ot = pout.tile([128, 3 * OUTW], F32, tag="ot")
                idv = ot[:, 0:3 * OUTW].rearrange("p (r c) -> p r c", r=3, c=OUTW)
                odv = (out[b * 128:(b + 1) * 128, :]
                       .rearrange("p (r c) -> p r c", r=3, c=OUTW))
                eye3 = iota[:, L:L + 9].rearrange("p (r c) -> p r c", r=3, c=3)
                # identity frame + zero tails straight from const tiles
                nc.gpsimd.dma_start(out=odv[:, :, 0:3], in_=eye3)
                if 3 + 3 * NP < OUTW:
                    tl = OUTW - (3 + 3 * NP)
                    ztv = zt[:, 0:tl].unsqueeze(1).broadcast_to([128, 3, tl])
                    nc.gpsimd.dma_start(out=odv[:, :, 3 + 3 * NP:], in_=ztv)

                if cb == 0 or ABLATE == "dma":
                    continue

                it = pin.tile([128, INW], F32, tag="it")
                nc.gpsimd.dma_start(out=it[:, :], in_=inp[b * 128:(b + 1) * 128, :])
                lens = it[:, 4 * L:4 * L + 1]

                # layout chosen so A-construction ops can pair adjacent
                # L-strided lanes: [sgn, m01, nsa, ca, sa, nca, sb, cb]
                tg = ptrig.tile([128, 8 * L], F32, tag="tg")
                sgn = tg[:, 0 * L:0 * L + NP]
                m01 = tg[:, 1 * L:1 * L + NP]
                nsa = tg[:, 2 * L:2 * L + NP]
                ca = tg[:, 3 * L:3 * L + NP]
                sa = tg[:, 4 * L:4 * L + NP]
                nca = tg[:, 5 * L:5 * L + NP]
                sb = tg[:, 6 * L:6 * L + NP]
                cb_ = tg[:, 7 * L:7 * L + NP]

                # absorber: first ACT write to the recycled trig tile carries
                # only the WAR dep (DVE/gpsimd readers of the old buffer)
                nc.scalar.activation(tg[:, 0:1], iota[:, 0:1], IDT)
                # ACT chain (first op joins the input DMA via the bias AP)
                nc.scalar.activation(sgn, iota[:, 0:NP], SIGN, bias=lens, scale=-1.0)
                nc.scalar.activation(m01, sgn, RELU)
                nc.scalar.activation(sa, it[:, 0:NP], SIN)
                nc.scalar.activation(ca, it[:, L:L + NP], SIN)
                nc.scalar.activation(nsa, it[:, 0:NP], SIN, scale=-1.0)
                nc.scalar.activation(nca, it[:, L:L + NP], SIN, scale=-1.0)
                nc.scalar.activation(sb, it[:, 2 * L:2 * L + NP], SIN)
                nc.scalar.activation(cb_, it[:, 3 * L:3 * L + NP], SIN)

                # A tile: [pos][c][m] (slot e = c*3+m), masked entries.
                # Paired ops: out slots with uniform stride, in0 = two adjacent
                # L-strided trig lanes, in1 broadcast over the pair.
                A = pwork.tile([128, 9 * L], F32, tag="A")
                Av = A[:, 0:9 * NP].rearrange("p (s e) -> p s e", s=NP, e=9)
                Ab = A[:, 0:9 * NP].rearrange("p (s a b) -> p s a b",
                                              s=NP, a=3, b=3)
                g = nc.vector

                def lanes2(base):
                    return (tg[:, base * L:(base + 2) * L]
                            .rearrange("p (e x) -> p x e", e=2, x=L)[:, 0:NP])

                def bcast2(other):
                    return other.unsqueeze(2).broadcast_to([128, NP, 2])

                # (ca,sa)*m01 -> e0,e1 ; (sb,cb)*m01 -> e5,e8
                g.tensor_tensor(out=Ab[:, :, 0, 0:2], in0=lanes2(3),
                                in1=bcast2(m01), op=MULT)
                g.tensor_tensor(out=Ab[:, :, 1:3, 2], in0=lanes2(6),
                                in1=bcast2(m01), op=MULT)
                g.memset(Av[:, :, 2], 0.0)
                # (nsa,ca)*cb_m -> e3,e4 ; (sa,nca)*sb_m -> e6,e7
                g.tensor_tensor(out=Ab[:, :, 1, 0:2], in0=lanes2(2),
                                in1=bcast2(Av[:, :, 8]), op=MULT)
                g.tensor_tensor(out=Ab[:, :, 2, 0:2], in0=lanes2(4),
                                in1=bcast2(Av[:, :, 5]), op=MULT)

                # ---- phase 1: local prefix scans, all chunks in parallel ----
                # P layout: [ch][r][jj][c]  (strides 9S / 3S / 3 / 1)
                P = pwork.tile([128, 9 * L], F32, tag="P")
                P5 = P[:, 0:9 * NP].rearrange("p (ch r jj c) -> p ch r jj c",
                                              ch=cb, r=3, jj=S, c=3)
                A5 = A[:, 0:9 * NP].rearrange("p (ch jj c m) -> p ch jj c m",
                                              ch=cb, jj=S, c=3, m=3)
                prods = pwork.tile([128, 27 * 64], F32, tag="prods")
                pr5 = prods[:, 0:27 * cb].rearrange("p (ch r c m) -> p ch r c m",
                                                    ch=cb, r=3, c=3, m=3)
                # step 0: P[ch][r][0][c] = A_entry[r][c] (A holds [c][m=r])
                nc.scalar.activation(P5[:, :, :, 0, :],
                                     A5[:, :, 0].transpose([0, 1, 3, 2]),
                                     IDT)
                for jj in range(0 if ABLATE == "aonly" else 1, S):
                    if ABLATE == "aonly":
                        break
                    in0 = (P5[:, :, :, jj - 1, :].unsqueeze(3)
                           .broadcast_to([128, cb, 3, 3, 3]))
                    in1 = (A5[:, :, jj].unsqueeze(2)
                           .broadcast_to([128, cb, 3, 3, 3]))
                    nc.vector.tensor_tensor(out=pr5, in0=in0, in1=in1, op=MULT)
                    nc.vector.tensor_tensor(out=P5[:, :, :, jj, :],
                                            in0=pr5[:, :, :, :, 0],
                                            in1=pr5[:, :, :, :, 1], op=ADD)
                    nc.vector.tensor_tensor(out=P5[:, :, :, jj, 1:3],
                                            in0=P5[:, :, :, jj, 1:3],
                                            in1=pr5[:, :, :, 1:3, 2], op=ADD)

                if cb > 1 and ABLATE not in ("noph23", "aonly"):
                    # ---- phase 2: Hillis-Steele scan over chunk carries ----
                    cA = pwork.tile([128, 9 * 32], F32, tag="cA")
                    cB = pwork.tile([128, 9 * 32], F32, tag="cB")
                    cph = pwork.tile([128, 27 * 32], F32, tag="cph")
                    # init: carries = chunk-final local products
                    a4 = cA[:, 0:9 * cb].rearrange("p (ch r c) -> p ch r c",
                                                   ch=cb, r=3, c=3)
                    nc.scalar.activation(a4, P5[:, :, :, S - 1, :], IDT)
                    cT = pwork.tile([128, 9 * 32], F32, tag="cT")
                    srcT, dstT = cA, cB
                    d = 1
                    while d < cb:
                        s4 = srcT[:, 0:9 * cb].rearrange("p (ch r c) -> p ch r c",
                                                         ch=cb, r=3, c=3)
                        d4 = dstT[:, 0:9 * cb].rearrange("p (ch r c) -> p ch r c",
                                                         ch=cb, r=3, c=3)
                        n = cb - d
                        # transposed copy of the RIGHT operands: cT[ch][c][m]
                        t4 = cT[:, 0:9 * n].rearrange("p (ch c m) -> p ch c m",
                                                      ch=n, c=3, m=3)
                        nc.vector.tensor_copy(out=t4,
                                              in_=s4[:, d:cb].transpose([0, 1, 3, 2]))
                        ph5 = cph[:, 0:27 * n].rearrange(
                            "p (ch r c m) -> p ch r c m", ch=n, r=3, c=3, m=3)
                        in0 = (s4[:, 0:n].unsqueeze(3)
                               .broadcast_to([128, n, 3, 3, 3]))
                        in1 = (t4.unsqueeze(2)
                               .broadcast_to([128, n, 3, 3, 3]))
                        nc.vector.tensor_tensor(out=ph5, in0=in0, in1=in1, op=MULT)
                        nc.vector.tensor_tensor(out=d4[:, d:cb],
                                                in0=ph5[:, :, :, :, 0],
                                                in1=ph5[:, :, :, :, 1], op=ADD)
                        nc.vector.tensor_tensor(out=d4[:, d:cb],
                                                in0=d4[:, d:cb],
                                                in1=ph5[:, :, :, :, 2], op=ADD)
                        nc.vector.tensor_copy(out=d4[:, 0:d], in_=s4[:, 0:d])
                        srcT, dstT = dstT, srcT
                        d *= 2
                    c4 = srcT[:, 0:9 * cb].rearrange("p (ch r c) -> p ch r c",
                                                     ch=cb, r=3, c=3)

                    # ---- phase 3: apply carries, write final layout ----
                    # p3all[r][ch][jj][c][m]
                    p3 = pwork.tile([128, 27 * L], F32, tag="p3")
                    NE = 9 * S * (cb - 1)
                    JC = 3 * S
                    p36 = p3[:, 0:3 * NE].rearrange(
                        "p (r ch jj c m) -> p r ch jj c m",
                        r=3, ch=cb - 1, jj=S, c=3, m=3)
                    for r in range(3):
                        p3m = p3[:, r * NE:(r + 1) * NE].rearrange(
                            "p (ch jc m) -> p ch jc m", ch=cb - 1, jc=JC, m=3)
                        in0 = (c4[:, 0:cb - 1, r, :].unsqueeze(2)
                               .broadcast_to([128, cb - 1, JC, 3]))
                        in1 = P5[:, 1:cb].rearrange("p ch m jj c -> p ch (jj c) m")
                        nc.vector.tensor_tensor(out=p3m, in0=in0, in1=in1, op=MULT)
                    ovr = (ot[:, 0:3 * OUTW]
                           .rearrange("p (r c) -> p r c", r=3, c=OUTW)
                           [:, :, 3 + 3 * S:3 + 3 * NP]
                           .rearrange("p r (ch jj c) -> p r ch jj c",
                                      ch=cb - 1, jj=S, c=3))
                    nc.vector.tensor_tensor(out=ovr, in0=p36[:, :, :, :, :, 0],
                                            in1=p36[:, :, :, :, :, 1], op=ADD)
                    nc.vector.tensor_tensor(out=ovr, in0=ovr,
                                            in1=p36[:, :, :, :, :, 2], op=ADD)

                # chunk 0 rows straight from P (one op for all 3 rows)
                ov0 = (ot[:, 0:3 * OUTW]
                       .rearrange("p (r c) -> p r c", r=3, c=OUTW)
                       [:, :, 3:3 + 3 * S]
                       .rearrange("p r (jj c) -> p r jj c", jj=S, c=3))
                nc.vector.tensor_copy(out=ov0, in_=P5[:, 0])

                nc.gpsimd.dma_start(out=odv[:, :, 3:3 + 3 * NP],
                                    in_=idv[:, :, 3:3 + 3 * NP])

    nc.finalize()
    return nc


def _wrap(x):
    return x - (2.0 * np.pi) * np.round(x / (2.0 * np.pi))


def prepare(input, angles_length):
    """Host-side prep: sort/stripe/wrap inputs, build (cached) Bass program."""
    input = np.asarray(input, dtype=np.float32)
    lens = np.asarray(angles_length).astype(np.int64)

    order = np.argsort(lens, kind="stable")
    in_maps = []
    core_lens = []
    for k in range(NCORES):
        idx = order[k::NCORES]
        core_lens.append(lens[idx])
        a = input[idx, 0, :]
        bta = input[idx, 1, :]
        arr = np.empty((BPC, INW), dtype=np.float32)
        arr[:, 0:L] = _wrap(a)
        arr[:, L:2 * L] = _wrap(a + np.pi / 2)
        arr[:, 2 * L:3 * L] = _wrap(bta)
        arr[:, 3 * L:4 * L] = _wrap(bta + np.pi / 2)
        arr[:, 4 * L] = core_lens[k] + 0.5
        arr[:, 4 * L + 1:] = 0.0
        in_maps.append({"inp": arr})

    iota = np.concatenate([
        np.broadcast_to(np.arange(1, L + 1, dtype=np.float32), (128, L)),
        np.broadcast_to(np.eye(3, dtype=np.float32).reshape(9), (128, 9)),
    ], axis=1).copy()
    for m in in_maps:
        m["cst"] = iota

    # per-block (S, C) plans from the max length across cores
    plans = []
    for b_ in range(NBLK):
        mx = max(int(core_lens[k][(b_ + 1) * 128 - 1]) for k in range(NCORES))
        plans.append(_plan_block(mx))
    key = tuple(plans)
    if key not in _CACHE:
        _CACHE[key] = _build(plans)
    nc = _CACHE[key]
    return nc, in_maps, order


def kernel(input, angles_length):
    nc, in_maps, order = prepare(input, angles_length)
    res = run_bass_kernel_spmd(nc, in_maps, core_ids=list(range(NCORES)))
    full = np.empty((B, 3, OUTW), dtype=np.float32)
    for k in range(NCORES):
        idx = order[k::NCORES]
        full[idx] = res.results[k]["out"].reshape(BPC, 3, OUTW)
    return full



# revision 3
# speedup vs baseline: 1.3113x; 1.3113x over previous
"""Angles2BasisDihedral Trainium2 kernel (8 NeuronCores, data-parallel).

Math: per sample b with angles alpha/beta (L=512), per-position rotation
  A_j = Rz(alpha_j) @ Rx(beta_j)  (3x3), cumulative M_p = A_1 @ ... @ A_p,
  output[b, r, 3p+c] = M_p[r][c] for p=0..L (M_0 = I), zeroed for p > len_b.

Device strategy (per core, 2048 samples = 16 blocks of 128 partitions):
  - host pre-wraps angles into [-pi, pi] (ACT Sin table is only accurate there)
    and pre-sorts samples by length, striped across the 8 cores; per-block
    chunk counts are baked into the instruction stream at build time.
  - masking is folded into the A entries: A_j = 0 for j > len_b, which makes
    every masked prefix product exactly zero.
  - chunked scan: C<=16 chunks of S=32 along the chain; phase 1 computes local
    prefixes for all chunks in parallel (batch in partitions, chunks in the
    free dim), phase 2 ripples the 3x3 carries across chunks, phase 3 applies
    carries to all local prefixes and writes rows in the final output layout.
  - engine-linear dataflow DMA -> ACT -> DVE -> DMA keeps every instruction at
    <=1 cross-engine semaphore wait (TRN2 ISA limit).
"""
import math
import os
import numpy as np

ABLATE = os.environ.get("KERNEL_ABLATE", "")
REPS = int(os.environ.get("KERNEL_REPS", "1"))

import concourse.bacc as bacc
import concourse.mybir as mybir
from concourse.bass_utils import run_bass_kernel_spmd
from concourse.tile import TileContext

B, L = 16384, 512
NCORES = 8
BPC = B // NCORES            # samples per core (2048)
NBLK = BPC // 128            # 16 blocks of 128 partitions
OUTW = 3 * (L + 1)           # 1539 columns per row
INW = 4 * L + 8              # wa | wac | wb | wbc | len+0.5 | pad


def _plan_block(maxlen):
    """(S, C) for this block's max length. Narrow chained DVE ops are far more
    expensive on silicon than the naive 58+FD model, so keep S=32 fixed."""
    if maxlen <= 0:
        return (32, 0)
    return (32, -(-maxlen // 32))
F32 = mybir.dt.float32
ADD = mybir.AluOpType.add
MULT = mybir.AluOpType.mult

LAST_EXEC_NS = None
_CACHE = {}


def _build(plans):
    """Build the Bass program. `plans` is a list of (S, C) per block."""
    nc = bacc.Bacc("TRN2", target_bir_lowering=False)
    # const needed for activation scale=-1.0
    t = nc.alloc_sbuf_tensor("const-f32-neg1", [128, 1], F32)
    nc.gpsimd.memset(t.ap(), -1.0)
    nc.const_aps.aps[(F32, -1.0)] = t.ap()
    nc.all_engine_barrier()

    inp = nc.declare_dram_parameter("inp", [BPC, INW], F32, isOutput=False)
    cst = nc.declare_dram_parameter("cst", [128, L + 9], F32, isOutput=False)
    out = nc.declare_dram_parameter("out", [BPC, 3 * OUTW], F32, isOutput=True)

    SIN = mybir.ActivationFunctionType.Sin
    SIGN = mybir.ActivationFunctionType.Sign
    RELU = mybir.ActivationFunctionType.Relu
    IDT = mybir.ActivationFunctionType.Identity

    with TileContext(nc) as tc:
        with (
            tc.tile_pool(name="pcst", bufs=1) as pcst,
            tc.tile_pool(name="pin", bufs=2) as pin,
            tc.tile_pool(name="ptrig", bufs=2) as ptrig,
            tc.tile_pool(name="pwork", bufs=1) as pwork,
            tc.tile_pool(name="pout", bufs=2) as pout,
        ):
            iota = pcst.tile([128, L + 9], F32)
            nc.gpsimd.dma_start(out=iota[:, :], in_=cst[:, :])
            # ACT warmup: absorb the const-DMA semaphore into ACT's clock
            warm = pcst.tile([128, 1], F32)
            nc.scalar.activation(warm[:, :], iota[:, 0:1], IDT)
            # zero tile: tails/identity are DMA'd to DRAM, not memset on DVE
            zt = pcst.tile([128, 1536], F32)
            nc.vector.memset(zt[:, :], 0.0)

            for b in range(NBLK * REPS):
                b = b % NBLK
                S, cb = plans[b]
                NP = cb * S
                ot = pout.tile([128, 3 * OUTW], F32, tag="ot")
                idv = ot[:, 0:3 * OUTW].rearrange("p (r c) -> p r c", r=3, c=OUTW)
                odv = (out[b * 128:(b + 1) * 128, :]
                       .rearrange("p (r c) -> p r c", r=3, c=OUTW))
                eye3 = iota[:, L:L + 9].rearrange("p (r c) -> p r c", r=3, c=3)
                # identity frame + zero tails straight from const tiles
                nc.gpsimd.dma_start(out=odv[:, :, 0:3], in_=eye3)
                if 3 + 3 * NP < OUTW:
                    tl = OUTW - (3 + 3 * NP)
                    ztv = zt[:, 0:tl].unsqueeze(1).broadcast_to([128, 3, tl])
                    nc.gpsimd.dma_start(out=odv[:, :, 3 + 3 * NP:], in_=ztv)

                if cb == 0 or ABLATE == "dma":
                    continue

                it = pin.tile([128, INW], F32, tag="it")
                nc.gpsimd.dma_start(out=it[:, :], in_=inp[b * 128:(b + 1) * 128, :])
                lens = it[:, 4 * L:4 * L + 1]

                # layout chosen so A-construction ops can pair adjacent
                # L-strided lanes: [sgn, m01, nsa, ca, sa, nca, sb, cb]
                tg = ptrig.tile([128, 8 * L], F32, tag="tg")
                sgn = tg[:, 0 * L:0 * L + NP]
                m01 = tg[:, 1 * L:1 * L + NP]
                nsa = tg[:, 2 * L:2 * L + NP]
                ca = tg[:, 3 * L:3 * L + NP]
                sa = tg[:, 4 * L:4 * L + NP]
                nca = tg[:, 5 * L:5 * L + NP]
                sb = tg[:, 6 * L:6 * L + NP]
                cb_ = tg[:, 7 * L:7 * L + NP]

                # absorber: first ACT write to the recycled trig tile carries
                # only the WAR dep (DVE/gpsimd readers of the old buffer)
                nc.scalar.activation(tg[:, 0:1], iota[:, 0:1], IDT)
                # ACT chain (first op joins the input DMA via the bias AP)
                nc.scalar.activation(sgn, iota[:, 0:NP], SIGN, bias=lens, scale=-1.0)
                nc.scalar.activation(m01, sgn, RELU)
                nc.scalar.activation(sa, it[:, 0:NP], SIN)
                nc.scalar.activation(ca, it[:, L:L + NP], SIN)
                nc.scalar.activation(nsa, it[:, 0:NP], SIN, scale=-1.0)
                nc.scalar.activation(nca, it[:, L:L + NP], SIN, scale=-1.0)
                nc.scalar.activation(sb, it[:, 2 * L:2 * L + NP], SIN)
                nc.scalar.activation(cb_, it[:, 3 * L:3 * L + NP], SIN)

                # A tile: [pos][c][m] (slot e = c*3+m), masked entries.
                # Paired ops: out slots with uniform stride, in0 = two adjacent
                # L-strided trig lanes, in1 broadcast over the pair.
                A = pwork.tile([128, 9 * L], F32, tag="A")
                Av = A[:, 0:9 * NP].rearrange("p (s e) -> p s e", s=NP, e=9)
                Ab = A[:, 0:9 * NP].rearrange("p (s a b) -> p s a b",
                                              s=NP, a=3, b=3)
                g = nc.vector

                def lanes2(base):
                    return (tg[:, base * L:(base + 2) * L]
                            .rearrange("p (e x) -> p x e", e=2, x=L)[:, 0:NP])

                def bcast2(other):
                    return other.unsqueeze(2).broadcast_to([128, NP, 2])

                # (ca,sa)*m01 -> e0,e1 ; (sb,cb)*m01 -> e5,e8
                g.tensor_tensor(out=Ab[:, :, 0, 0:2], in0=lanes2(3),
                                in1=bcast2(m01), op=MULT)
                g.tensor_tensor(out=Ab[:, :, 1:3, 2], in0=lanes2(6),
                                in1=bcast2(m01), op=MULT)
                g.memset(Av[:, :, 2], 0.0)
                # (nsa,ca)*cb_m -> e3,e4 ; (sa,nca)*sb_m -> e6,e7
                g.tensor_tensor(out=Ab[:, :, 1, 0:2], in0=lanes2(2),
                                in1=bcast2(Av[:, :, 8]), op=MULT)
                g.tensor_tensor(out=Ab[:, :, 2, 0:2], in0=lanes2(4),
                                in1=bcast2(Av[:, :, 5]), op=MULT)

                # ---- phase 1: local prefix scans, all chunks in parallel ----
                # P layout: [ch][r][jj][c]  (strides 9S / 3S / 3 / 1)
                P = pwork.tile([128, 9 * L], F32, tag="P")
                P5 = P[:, 0:9 * NP].rearrange("p (ch r jj c) -> p ch r jj c",
                                              ch=cb, r=3, jj=S, c=3)
                A5 = A[:, 0:9 * NP].rearrange("p (ch jj c m) -> p ch jj c m",
                                              ch=cb, jj=S, c=3, m=3)
                prods = pwork.tile([128, 27 * 64], F32, tag="prods")
                pr5 = prods[:, 0:27 * cb].rearrange("p (ch r c m) -> p ch r c m",
                                                    ch=cb, r=3, c=3, m=3)
                # step 0: P[ch][r][0][c] = A_entry[r][c] (A holds [c][m=r])
                nc.scalar.activation(P5[:, :, :, 0, :],
                                     A5[:, :, 0].transpose([0, 1, 3, 2]),
                                     IDT)
                for jj in range(0 if ABLATE == "aonly" else 1, S):
                    if ABLATE == "aonly":
                        break
                    in0 = (P5[:, :, :, jj - 1, :].unsqueeze(3)
                           .broadcast_to([128, cb, 3, 3, 3]))
                    in1 = (A5[:, :, jj].unsqueeze(2)
                           .broadcast_to([128, cb, 3, 3, 3]))
                    nc.vector.tensor_tensor(out=pr5, in0=in0, in1=in1, op=MULT)
                    nc.vector.tensor_tensor(out=P5[:, :, :, jj, :],
                                            in0=pr5[:, :, :, :, 0],
                                            in1=pr5[:, :, :, :, 1], op=ADD)
                    nc.vector.tensor_tensor(out=P5[:, :, :, jj, 1:3],
                                            in0=P5[:, :, :, jj, 1:3],
                                            in1=pr5[:, :, :, 1:3, 2], op=ADD)

                if cb > 1 and ABLATE not in ("noph23", "aonly"):
                    # ---- phase 2: Hillis-Steele scan over chunk carries ----
                    cA = pwork.tile([128, 9 * 32], F32, tag="cA")
                    cB = pwork.tile([128, 9 * 32], F32, tag="cB")
                    cph = pwork.tile([128, 27 * 32], F32, tag="cph")
                    # init: carries = chunk-final local products
                    a4 = cA[:, 0:9 * cb].rearrange("p (ch r c) -> p ch r c",
                                                   ch=cb, r=3, c=3)
                    nc.scalar.activation(a4, P5[:, :, :, S - 1, :], IDT)
                    cT = pwork.tile([128, 9 * 32], F32, tag="cT")
                    srcT, dstT = cA, cB
                    d = 1
                    while d < cb:
                        s4 = srcT[:, 0:9 * cb].rearrange("p (ch r c) -> p ch r c",
                                                         ch=cb, r=3, c=3)
                        d4 = dstT[:, 0:9 * cb].rearrange("p (ch r c) -> p ch r c",
                                                         ch=cb, r=3, c=3)
                        n = cb - d
                        # transposed copy of the RIGHT operands: cT[ch][c][m]
                        t4 = cT[:, 0:9 * n].rearrange("p (ch c m) -> p ch c m",
                                                      ch=n, c=3, m=3)
                        nc.vector.tensor_copy(out=t4,
                                              in_=s4[:, d:cb].transpose([0, 1, 3, 2]))
                        ph5 = cph[:, 0:27 * n].rearrange(
                            "p (ch r c m) -> p ch r c m", ch=n, r=3, c=3, m=3)
                        in0 = (s4[:, 0:n].unsqueeze(3)
                               .broadcast_to([128, n, 3, 3, 3]))
                        in1 = (t4.unsqueeze(2)
                               .broadcast_to([128, n, 3, 3, 3]))
                        nc.vector.tensor_tensor(out=ph5, in0=in0, in1=in1, op=MULT)
                        nc.vector.tensor_tensor(out=d4[:, d:cb],
                                                in0=ph5[:, :, :, :, 0],
                                                in1=ph5[:, :, :, :, 1], op=ADD)
                        nc.vector.tensor_tensor(out=d4[:, d:cb],
                                                in0=d4[:, d:cb],
                                                in1=ph5[:, :, :, :, 2], op=ADD)
                        nc.vector.tensor_copy(out=d4[:, 0:d], in_=s4[:, 0:d])
                        srcT, dstT = dstT, srcT
                        d *= 2
                    c4 = srcT[:, 0:9 * cb].rearrange("p (ch r c) -> p ch r c",
                                                     ch=cb, r=3, c=3)

                    # ---- phase 3: apply carries, write final layout ----
                    # p3all[r][ch][jj][c][m]
                    p3 = pwork.tile([128, 27 * L], F32, tag="p3")
                    NE = 9 * S * (cb - 1)
                    JC = 3 * S
                    p36 = p3[:, 0:3 * NE].rearrange(
                        "p (r ch jj c m) -> p r ch jj c m",
                        r=3, ch=cb - 1, jj=S, c=3, m=3)
                    for r in range(3):
                        p3m = p3[:, r * NE:(r + 1) * NE].rearrange(
                            "p (ch jc m) -> p ch jc m", ch=cb - 1, jc=JC, m=3)
                        in0 = (c4[:, 0:cb - 1, r, :].unsqueeze(2)
                               .broadcast_to([128, cb - 1, JC, 3]))
                        in1 = P5[:, 1:cb].rearrange("p ch m jj c -> p ch (jj c) m")
                        nc.vector.tensor_tensor(out=p3m, in0=in0, in1=in1, op=MULT)
                    ovr = (ot[:, 0:3 * OUTW]
                           .rearrange("p (r c) -> p r c", r=3, c=OUTW)
                           [:, :, 3 + 3 * S:3 + 3 * NP]
                           .rearrange("p r (ch jj c) -> p r ch jj c",
                                      ch=cb - 1, jj=S, c=3))
                    nc.vector.tensor_tensor(out=ovr, in0=p36[:, :, :, :, :, 0],
                                            in1=p36[:, :, :, :, :, 1], op=ADD)
                    nc.vector.tensor_tensor(out=ovr, in0=ovr,
                                            in1=p36[:, :, :, :, :, 2], op=ADD)

                # chunk 0 rows straight from P (one op for all 3 rows)
                ov0 = (ot[:, 0:3 * OUTW]
                       .rearrange("p (r c) -> p r c", r=3, c=OUTW)
                       [:, :, 3:3 + 3 * S]
                       .rearrange("p r (jj c) -> p r jj c", jj=S, c=3))
                nc.vector.tensor_copy(out=ov0, in_=P5[:, 0])

                nc.gpsimd.dma_start(out=odv[:, :, 3:3 + 3 * NP],
                                    in_=idv[:, :, 3:3 + 3 * NP])

    nc.finalize()
    return nc


def _wrap(x):
    return x - (2.0 * np.pi) * np.round(x / (2.0 * np.pi))


def prepare(input, angles_length):
    """Host-side prep: sort/stripe/wrap inputs, build (cached) Bass program."""
    input = np.asarray(input, dtype=np.float32)
    lens = np.asarray(angles_length).astype(np.int64)

    order = np.argsort(lens, kind="stable")
    in_maps = []
    core_lens = []
    for k in range(NCORES):
        idx = order[k::NCORES]
        core_lens.append(lens[idx])
        a = input[idx, 0, :]
        bta = input[idx, 1, :]
        arr = np.empty((BPC, INW), dtype=np.float32)
        arr[:, 0:L] = _wrap(a)
        arr[:, L:2 * L] = _wrap(a + np.pi / 2)
        arr[:, 2 * L:3 * L] = _wrap(bta)
        arr[:, 3 * L:4 * L] = _wrap(bta + np.pi / 2)
        arr[:, 4 * L] = core_lens[k] + 0.5
        arr[:, 4 * L + 1:] = 0.0
        in_maps.append({"inp": arr})

    iota = np.concatenate([
        np.broadcast_to(np.arange(1, L + 1, dtype=np.float32), (128, L)),
        np.broadcast_to(np.eye(3, dtype=np.float32).reshape(9), (128, 9)),
    ], axis=1).copy()
    for m in in_maps:
        m["cst"] = iota

    # per-block (S, C) plans from the max length across cores
    plans = []
    for b_ in range(NBLK):
        mx = max(int(core_lens[k][(b_ + 1) * 128 - 1]) for k in range(NCORES))
        plans.append(_plan_block(mx))
    key = tuple(plans)
    if key not in _CACHE:
        _CACHE[key] = _build(plans)
    nc = _CACHE[key]
    return nc, in_maps, order


def kernel(input, angles_length):
    nc, in_maps, order = prepare(input, angles_length)
    res = run_bass_kernel_spmd(nc, in_maps, core_ids=list(range(NCORES)))
    full = np.empty((B, 3, OUTW), dtype=np.float32)
    for k in range(NCORES):
        idx = order[k::NCORES]
        full[idx] = res.results[k]["out"].reshape(BPC, 3, OUTW)
    return full

